# revision 17
# baseline (speedup 1.0000x reference)
"""Trainium2 Bass kernel for nn_DecFormerT1 (dense transformer block).

Computation (see problem reference):
  x [2, 8, 128, 24, 24] ->
  qkv projections (+ sine pos embed on q,k) -> full softmax attention over
  n = t*h*w = 4608 -> residual -> channels-first LayerNorm -> grouped-conv
  3x3 FFN (128 -> 512 -> 128, 32 groups) with relu -> residual -> LayerNorm.

Sharding over 8 cores: core j handles batch j//4, query/FFN t-slice
[2*(j%4), 2*(j%4)+2).  K/V are recomputed per-core for the full sequence
(cheap: 2 x [4608,128]@[128,128]) so no collectives are needed.

On-chip layout is channels-first: activations live as [c=128 partitions,
positions] tiles.  Attention uses S^T blocks ([nk, nq], softmax along
partitions via ones-matmul rowsums), P@V accumulates O^T = V^T P^T per
nq-tile of 384, and the FFN does LayerNorm-over-partitions via GPSIMD
partition_all_reduce + grouped conv as 9 shifted matmuls on zero-padded
[c, 26*26] image tiles with block-diagonal dense weights.

Matmuls run in float32r (tf32-like, ~1e-3 rel err, 4x faster than fp32).
"""

from contextlib import ExitStack

import numpy as np

import concourse.bass as bass
import concourse.tile as tile
from concourse import bacc, mybir
from concourse.bass_utils import run_bass_kernel_spmd

F32 = mybir.dt.float32
F32R = mybir.dt.float32r

B, T, C, H, W = 2, 8, 128, 24, 24
HW = H * W  # 576
N = T * HW  # 4608
TPC = 2  # t per core
NQ = TPC * HW  # 1152
NCORES = 8
GROUPS = 32
CH = 4 * C  # 512
EPS = 1e-6
TEMP = 10000.0

NQT = 384  # nq tile for attention
NKB = N // 128  # 36 key blocks

ALU = mybir.AluOpType
ACTF = mybir.ActivationFunctionType


def _pos_embed_np() -> np.ndarray:
    """PositionEmbeddingSine3D closed form, [N, C] float32 (mirrors reference)."""
    npf = C // 2
    scale = 2.0 * np.pi

    def sine(coord, nf):
        dim_t = (TEMP ** (2.0 * (np.arange(nf) // 2).astype(np.float32) / nf)).astype(
            np.float32
        )
        p = coord[:, None] / dim_t  # [L, nf]
        return np.stack(
            [np.sin(p[:, 0::2]), np.cos(p[:, 1::2])], axis=-1
        ).reshape(coord.shape[0], nf)

    z = (np.arange(1, T + 1, dtype=np.float32) / np.float32(T + EPS)) * np.float32(
        scale
    )
    y = (np.arange(1, H + 1, dtype=np.float32) / np.float32(H + EPS)) * np.float32(
        scale
    )
    x = (np.arange(1, W + 1, dtype=np.float32) / np.float32(W + EPS)) * np.float32(
        scale
    )
    pz = sine(z, 2 * npf)  # [T, C]
    py = sine(y, npf)  # [H, npf]
    px = sine(x, npf)  # [W, npf]
    pos = np.empty((T, H, W, C), dtype=np.float32)
    pos[..., :npf] = py[None, :, None, :]
    pos[..., npf:] = px[None, None, :, :]
    pos = pos + pz[:, None, None, :]
    return pos.reshape(N, C)


def build_program() -> bacc.Bacc:
    nc = bacc.Bacc("TRN2", target_bir_lowering=False, debug=False, num_devices=NCORES)

    def din(name, shape, dt=F32):
        return nc.dram_tensor(name, shape, dt, kind="ExternalInput").ap()

    # per-core data
    xb_r = din("xb_r", [T, C, HW], F32R)  # batch for k/v
    xq_r = din("xq_r", [TPC, C, HW], F32R)  # slice for q
    xq_f = din("xq_f", [TPC, C, HW], F32)  # slice for residual
    posqb = din("posqb", [C, NQ])  # (pos_slice + bq)/sqrt(C)
    # shared
    poskb = din("poskb", [C, N])  # pos + bk
    wqT = din("wqT", [C, C], F32R)  # Wq.T / sqrt(C)
    wkT = din("wkT", [C, C], F32R)
    wvT = din("wvT", [C, C], F32R)
    bv = din("bv", [C, 1])
    id_r = din("id_r", [C, C], F32R)
    ones_r = din("ones_r", [C, 1], F32R)
    w1 = din("w1", [C, 9, 4, C], F32R)  # conv1 lhsT [ic, tap, occhunk, oc]
    b1 = din("b1", [C, 4])
    w2 = din("w2", [C, 9, 4, C], F32R)  # conv2 lhsT [icw, tap, icchunk, oc]
    zpad = din("zpad", [C, H, 2], F32R)  # zeros for image pad columns
    b2 = din("b2", [C, 1])
    n1w = din("n1w", [C, 1])
    n1b = din("n1b", [C, 1])
    n2w = din("n2w", [C, 1])
    n2b = din("n2b", [C, 1])

    out = nc.dram_tensor("out", [TPC, C, HW], F32, kind="ExternalOutput").ap()

    with tile.TileContext(nc) as tc, ExitStack() as octx:
        # long-lived pools (span attention phases)
        keep = octx.enter_context(tc.tile_pool(name="keep", bufs=1))
        consts = octx.enter_context(tc.tile_pool(name="consts", bufs=1))
        abctx = octx.enter_context(ExitStack())
        abpool = abctx.enter_context(tc.tile_pool(name="abpool", bufs=1))

        kT = abpool.tile([C, N], F32R)
        vb = abpool.tile([C, NKB, C], F32R)  # v blocks [nk, c] (transposed)
        qT = abpool.tile([C, NQ], F32R)
        xqf = keep.tile([C, TPC, HW], F32)
        nc.sync.dma_start(xqf, xq_f.rearrange("t c s -> c t s"))
        ot_sb = keep.tile([C, NQ], F32)  # attention out (normalized)

        onesr = consts.tile([C, 1], F32R)
        nc.sync.dma_start(onesr, ones_r)
        bvt = consts.tile([C, 1], F32)
        nc.sync.dma_start(bvt, bv)
        epst = consts.tile([C, 1], F32)
        nc.vector.memset(epst, EPS)

        # ---------------- phase A: projections + v transpose ----------------
        with ExitStack() as actx:
            apool = actx.enter_context(tc.tile_pool(name="apool", bufs=1))
            apsum = actx.enter_context(
                tc.tile_pool(name="apsum", bufs=4, space="PSUM")
            )
            tpsum = actx.enter_context(
                tc.tile_pool(name="tpsum", bufs=2, space="PSUM")
            )

            xb = apool.tile([C, T, HW], F32R)
            nc.sync.dma_start(xb, xb_r.rearrange("t c s -> c t s"))
            xq = apool.tile([C, TPC, HW], F32R)
            nc.sync.dma_start(xq, xq_r.rearrange("t c s -> c t s"))
            poskbt = apool.tile([C, N], F32)
            nc.sync.dma_start(poskbt, poskb)
            posqbt = apool.tile([C, NQ], F32)
            nc.sync.dma_start(posqbt, posqb)
            wq = apool.tile([C, C], F32R)
            nc.sync.dma_start(wq, wqT)
            wk = apool.tile([C, C], F32R)
            nc.sync.dma_start(wk, wkT)
            wv = apool.tile([C, C], F32R)
            nc.sync.dma_start(wv, wvT)
            idr = apool.tile([C, C], F32R)
            nc.sync.dma_start(idr, id_r)

            xbf = xb.rearrange("c t s -> c (t s)")
            vT = apool.tile([C, N], F32R)
            for i in range(N // 512):
                sl = bass.ts(i, 512)
                pk = apsum.tile([C, 512], F32, tag="proj")
                nc.tensor.matmul(pk, wk, xbf[:, sl], start=True, stop=True)
                nc.vector.tensor_tensor(
                    kT[:, sl], pk, poskbt[:, sl], op=ALU.add
                )
                pv = apsum.tile([C, 512], F32, tag="proj")
                nc.tensor.matmul(pv, wv, xbf[:, sl], start=True, stop=True)
                nc.vector.tensor_scalar(
                    out=vT[:, sl], in0=pv, scalar1=bvt, scalar2=None, op0=ALU.add
                )
            xqf_r = xq.rearrange("c t s -> c (t s)")
            for i in range(NQ // NQT):
                sl = bass.ts(i, NQT)
                pq = apsum.tile([C, NQT], F32, tag="proj")
                nc.tensor.matmul(pq, wq, xqf_r[:, sl], start=True, stop=True)
                nc.vector.tensor_tensor(
                    qT[:, sl], pq, posqbt[:, sl], op=ALU.add
                )
            # transpose v into [nk, c] blocks
            for nb in range(NKB):
                pt = tpsum.tile([C, C], F32R, tag="pt")
                nc.tensor.transpose(pt, vT[:, bass.ts(nb, C)], idr)
                nc.vector.tensor_copy(vb[:, nb, :], pt)

        # ---------------- phase B: attention ----------------
        with ExitStack() as bctx:
            ptpool = bctx.enter_context(tc.tile_pool(name="ptpool", bufs=3))
            spsum = bctx.enter_context(
                tc.tile_pool(name="spsum", bufs=2, space="PSUM")
            )
            opsum = bctx.enter_context(
                tc.tile_pool(name="opsum", bufs=1, space="PSUM")
            )
            rpsum = bctx.enter_context(
                tc.tile_pool(name="rpsum", bufs=1, space="PSUM")
            )
            npool = bctx.enter_context(tc.tile_pool(name="npool", bufs=2))

            for oq in range(NQ // NQT):
                qsl = bass.ts(oq, NQT)
                ot_ps = opsum.tile([C, NQT], F32, tag="ot")
                rs_ps = rpsum.tile([1, NQT], F32, tag="rs")
                for tri in range(NKB // 3):
                    st = spsum.tile([C, 3, 512], F32, tag="st")
                    for j in range(3):
                        nk = 3 * tri + j
                        nc.tensor.matmul(
                            st[:, j, 0:NQT],
                            kT[:, bass.ts(nk, C)],
                            qT[:, qsl],
                            start=True,
                            stop=True,
                        )
                    pt = ptpool.tile([C, 3, NQT], F32R, tag="pt")
                    nc.scalar.activation(pt, st[:, :, 0:NQT], ACTF.Exp)
                    for j in range(3):
                        nk = 3 * tri + j
                        nc.tensor.matmul(
                            ot_ps,
                            vb[:, nk, :],
                            pt[:, j, :],
                            start=(nk == 0),
                            stop=(nk == NKB - 1),
                        )
                        nc.tensor.matmul(
                            rs_ps,
                            onesr,
                            pt[:, j, :],
                            start=(nk == 0),
                            stop=(nk == NKB - 1),
                        )
                rinv = npool.tile([1, NQT], F32, tag="rinv")
                nc.vector.reciprocal(rinv, rs_ps)
                rb = npool.tile([C, NQT], F32, tag="rb")
                nc.gpsimd.partition_broadcast(rb, rinv)
                nc.vector.tensor_tensor(ot_sb[:, qsl], ot_ps, rb, op=ALU.mult)

        abctx.close()  # free kT/vb/qT before the FFN phase

        # ---------------- phase C: residual + LN1 + FFN + LN2 ----------------
        with ExitStack() as cctx:
            cpool = cctx.enter_context(tc.tile_pool(name="cpool", bufs=1))
            lnt = cctx.enter_context(tc.tile_pool(name="lnt", bufs=2))
            hidp = cctx.enter_context(tc.tile_pool(name="hidp", bufs=2))
            c1ps = cctx.enter_context(
                tc.tile_pool(name="c1ps", bufs=4, space="PSUM")
            )
            c2ps = cctx.enter_context(
                tc.tile_pool(name="c2ps", bufs=2, space="PSUM")
            )

            w1t = cpool.tile([C, 9, 4, C], F32R)
            nc.sync.dma_start(w1t, w1)
            w2t = cpool.tile([C, 9, 4, C], F32R)
            nc.sync.dma_start(w2t, w2)
            b1t = cpool.tile([C, 4], F32)
            nc.sync.dma_start(b1t, b1)
            b2t = cpool.tile([C, 1], F32)
            nc.sync.dma_start(b2t, b2)
            n1wt = cpool.tile([C, 1], F32)
            nc.sync.dma_start(n1wt, n1w)
            n1bt = cpool.tile([C, 1], F32)
            nc.sync.dma_start(n1bt, n1b)
            n2wt = cpool.tile([C, 1], F32)
            nc.sync.dma_start(n2wt, n2w)
            n2bt = cpool.tile([C, 1], F32)
            nc.sync.dma_start(n2bt, n2b)

            def layernorm(dst, src, wt, bt):
                """channels-first LN over partitions on [C, NQ] tiles.

                var*C = s2 - s1^2/C, with the final /C folded into the Sqrt
                activation's scale.  Tag rotation keeps SBUF use at 5 tiles.
                """
                s1 = lnt.tile([C, NQ], F32, tag="ln_b")
                nc.gpsimd.partition_all_reduce(
                    s1, src, channels=C, reduce_op=bass.bass_isa.ReduceOp.add
                )
                sq = lnt.tile([C, NQ], F32, tag="ln_a")
                nc.vector.tensor_tensor(sq, src, src, op=ALU.mult)
                s2 = lnt.tile([C, NQ], F32, tag="ln_c")
                nc.gpsimd.partition_all_reduce(
                    s2, sq, channels=C, reduce_op=bass.bass_isa.ReduceOp.add
                )
                s1sq = lnt.tile([C, NQ], F32, tag="ln_a")
                nc.vector.tensor_tensor(s1sq, s1, s1, op=ALU.mult)
                varp = lnt.tile([C, NQ], F32, tag="ln_d")
                nc.vector.scalar_tensor_tensor(
                    out=varp,
                    in0=s1sq,
                    scalar=-1.0 / C,
                    in1=s2,
                    op0=ALU.mult,
                    op1=ALU.add,
                )
                sd = lnt.tile([C, NQ], F32, tag="ln_c")
                nc.scalar.activation(sd, varp, ACTF.Sqrt, bias=epst, scale=1.0 / C)
                inv = lnt.tile([C, NQ], F32, tag="ln_d")
                nc.vector.reciprocal(inv, sd)
                yc = lnt.tile([C, NQ], F32, tag="ln_a")
                nc.vector.scalar_tensor_tensor(
                    out=yc,
                    in0=s1,
                    scalar=-1.0 / C,
                    in1=src,
                    op0=ALU.mult,
                    op1=ALU.add,
                )
                yn = lnt.tile([C, NQ], F32, tag="ln_b")
                nc.vector.tensor_tensor(yn, yc, inv, op=ALU.mult)
                nc.vector.tensor_scalar(
                    out=dst, in0=yn, scalar1=wt, scalar2=bt, op0=ALU.mult, op1=ALU.add
                )

            # y = attn_out + x ; y_ln = LN1(y)
            y_sb = cpool.tile([C, NQ], F32)
            nc.vector.tensor_tensor(
                y_sb, ot_sb, xqf.rearrange("c t s -> c (t s)"), op=ALU.add
            )
            y_ln = cpool.tile([C, NQ], F32)
            layernorm(y_ln, y_sb, n1wt, n1bt)

            z_in = cpool.tile([C, NQ], F32)
            zpt = cpool.tile([C, H, 2], F32R)
            nc.sync.dma_start(zpt, zpad)

            PW = W + 2  # column-padded image width (26)

            def pad_cols(t_ap):
                """Write the two zero pad columns (0 and PW-1) of a [C,H,PW] view."""
                v = t_ap.rearrange("c (h w) -> c h w", w=PW)
                nc.vector.tensor_copy(v[:, :, 0:1], zpt[:, :, 0:1])
                nc.vector.tensor_copy(v[:, :, PW - 1 : PW], zpt[:, :, 1:2])

            def row_window(half, tap):
                """Row-clipped SAME-conv window for one 3x3 tap on a 12-row half.

                Output rows are clipped (PSUM dst stays contiguous); columns
                always valid thanks to the zero pad columns.  has_written bits
                handle rows partially covered across taps (center tap covers
                every element).  Returns (out_rows, in_rows, in_cols).
                """
                dy, dx = tap // 3, tap % 3
                y0 = max(12 * half, 1 - dy)
                y1 = min(12 * half + 12, H + 1 - dy)
                return (
                    slice(y0 - 12 * half, y1 - 12 * half),
                    slice(y0 + dy - 1, y1 + dy - 1),
                    slice(dx, dx + W),
                )

            for img in range(TPC):
                isl = bass.ts(img, HW)
                yp = hidp.tile([C, H * PW], F32R, tag="ypad")
                pad_cols(yp)
                ypv = yp.rearrange("c (h w) -> c h w", w=PW)
                nc.vector.tensor_copy(
                    ypv[:, :, 1 : W + 1],
                    y_ln[:, isl].rearrange("c (h w) -> c h w", w=W),
                )
                hids = []
                for ch in range(4):
                    hid = hidp.tile([C, H * PW], F32R, tag=f"hid{ch}")
                    pad_cols(hid)
                    hids.append(hid)
                # conv1 + bias + relu -> hids
                for ch in range(4):
                    for half in range(2):
                        ps = c1ps.tile([C, 288], F32, tag="c1")
                        psv = ps.rearrange("c (h w) -> c h w", w=W)
                        for tap in range(9):
                            orows, irows, icols = row_window(half, tap)
                            nc.tensor.matmul(
                                psv[:, orows, :],
                                w1t[:, tap, ch, :],
                                ypv[:, irows, icols],
                                start=(tap == 0),
                                stop=(tap == 8),
                            )
                        hv = hids[ch].rearrange("c (h w) -> c h w", w=PW)
                        nc.vector.tensor_scalar(
                            out=hv[:, bass.ts(half, 12), 1 : W + 1],
                            in0=ps,
                            scalar1=b1t[:, ch : ch + 1],
                            scalar2=0.0,
                            op0=ALU.add,
                            op1=ALU.max,
                        )
                # conv2 + bias + residual(y_ln)
                for half in range(2):
                    ps2 = c2ps.tile([C, 288], F32, tag="c2")
                    ps2v = ps2.rearrange("c (h w) -> c h w", w=W)
                    for ch in range(4):
                        hv = hids[ch].rearrange("c (h w) -> c h w", w=PW)
                        for tap in range(9):
                            orows, irows, icols = row_window(half, tap)
                            nc.tensor.matmul(
                                ps2v[:, orows, :],
                                w2t[:, tap, ch, :],
                                hv[:, irows, icols],
                                start=(ch == 0 and tap == 0),
                                stop=(ch == 3 and tap == 8),
                            )
                    hsl = bass.ds(img * HW + half * 288, 288)
                    nc.vector.scalar_tensor_tensor(
                        out=z_in[:, hsl],
                        in0=ps2,
                        scalar=b2t,
                        in1=y_ln[:, hsl],
                        op0=ALU.add,
                        op1=ALU.add,
                    )

            z_out = cpool.tile([C, NQ], F32)
            layernorm(z_out, z_in, n2wt, n2bt)
            nc.sync.dma_start(
                out.rearrange("t c s -> c t s"),
                z_out.rearrange("c (t s) -> c t s", t=TPC),
            )

    nc.compile()
    return nc


_CACHED_NC = None


def _get_nc():
    global _CACHED_NC
    if _CACHED_NC is None:
        _CACHED_NC = build_program()
    return _CACHED_NC


def make_in_maps(inputs: dict) -> list[dict]:
    x = np.asarray(inputs["x"], dtype=np.float32)
    Wq = np.asarray(inputs["Wq"], dtype=np.float32)
    bq = np.asarray(inputs["bq"], dtype=np.float32)
    Wk = np.asarray(inputs["Wk"], dtype=np.float32)
    bk = np.asarray(inputs["bk"], dtype=np.float32)
    Wv = np.asarray(inputs["Wv"], dtype=np.float32)
    bv_ = np.asarray(inputs["bv"], dtype=np.float32)
    conv1_w = np.asarray(inputs["conv1_w"], dtype=np.float32)
    conv1_b = np.asarray(inputs["conv1_b"], dtype=np.float32)
    conv2_w = np.asarray(inputs["conv2_w"], dtype=np.float32)
    conv2_b = np.asarray(inputs["conv2_b"], dtype=np.float32)

    pos = _pos_embed_np()  # [N, C]
    isq = np.float32(1.0 / np.sqrt(np.float32(C)))

    poskb_np = np.ascontiguousarray((pos + bk[None, :]).T)  # [C, N]
    posq_all = ((pos + bq[None, :]) * isq).T  # [C, N]

    # conv1 block-diagonal dense lhsT [ic, tap, chunk, oc]
    w1_np = np.zeros((C, 9, 4, C), dtype=np.float32)
    for oc in range(CH):
        g = oc // (CH // GROUPS)  # 16 oc per group
        for icg in range(C // GROUPS):  # 4 ic per group
            ic = g * (C // GROUPS) + icg
            w1_np[ic, :, oc // C, oc % C] = conv1_w[oc, icg].reshape(9)
    # conv2 block-diagonal dense lhsT [ic%128, tap, ic//128, oc]
    w2_np = np.zeros((C, 9, 4, C), dtype=np.float32)
    for oc in range(C):
        g = oc // (C // GROUPS)
        for icg in range(CH // GROUPS):  # 16 ic per group
            ic = g * (CH // GROUPS) + icg
            w2_np[ic % C, :, ic // C, oc] = conv2_w[oc, icg].reshape(9)

    shared = {
        "poskb": poskb_np,
        "wqT": np.ascontiguousarray(Wq.T) * isq,
        "wkT": np.ascontiguousarray(Wk.T),
        "wvT": np.ascontiguousarray(Wv.T),
        "bv": bv_[:, None],
        "id_r": np.eye(C, dtype=np.float32),
        "ones_r": np.ones((C, 1), dtype=np.float32),
        "zpad": np.zeros((C, H, 2), dtype=np.float32),
        "w1": w1_np,
        "b1": np.ascontiguousarray(conv1_b.reshape(4, C).T),
        "b2": conv2_b[:, None],
        "n1w": np.asarray(inputs["norm1_w"], dtype=np.float32)[:, None],
        "n1b": np.asarray(inputs["norm1_b"], dtype=np.float32)[:, None],
        "n2w": np.asarray(inputs["norm2_w"], dtype=np.float32)[:, None],
        "n2b": np.asarray(inputs["norm2_b"], dtype=np.float32)[:, None],
        "w2": w2_np,
    }

    in_maps = []
    for j in range(NCORES):
        bi = j // 4
        t0 = TPC * (j % 4)
        xb_np = np.ascontiguousarray(x[bi].reshape(T, C, HW))
        xq_np = np.ascontiguousarray(x[bi, t0 : t0 + TPC].reshape(TPC, C, HW))
        posq_np = np.ascontiguousarray(
            posq_all[:, t0 * HW : (t0 + TPC) * HW]
        )
        in_maps.append(
            {
                "xb_r": xb_np,
                "xq_r": xq_np,
                "xq_f": xq_np,
                "posqb": posq_np,
                **shared,
            }
        )
    return in_maps


def gather_output(results: list[dict]) -> np.ndarray:
    out = np.empty((B, T, C, H, W), dtype=np.float32)
    for j in range(NCORES):
        bi = j // 4
        t0 = TPC * (j % 4)
        out[bi, t0 : t0 + TPC] = results[j]["out"].reshape(TPC, C, H, W)
    return out


def kernel(**inputs) -> np.ndarray:
    nc = _get_nc()
    in_maps = make_in_maps(inputs)
    res = run_bass_kernel_spmd(nc, in_maps, list(range(NCORES)))
    return gather_output(res.results)
